# revision 1
# baseline (speedup 1.0000x reference)
"""Trainium2 Bass kernel for nn_ChannelWiseConv (depthwise conv stack + KAN head).

Strategy (per core, pure data parallelism over batch):
  - 256 images/core. Each stride-2 depthwise 3x3 conv is computed as 3 PE
    matmuls accumulating in PSUM: contraction over input rows with per-channel
    banded weight matrices (one per kernel-column tap); the column subsampling
    is expressed in the moving-operand access pattern. Channels are blocked
    (2/4/8/13 per matmul as planes shrink) to keep K near 128.
  - bias+relu fused into one ACT/DVE op (PSUM -> SBUF) that also scatters into
    the next layer's channel-blocked input layout.
  - KAN layers: B-spline basis via unnormalized Cox-de-Boor on a uniform grid
    (all denominators folded into host-precomputed coefficient matrices), then
    one PE matmul per KAN layer over [silu(x); basis; 1] stacks.
  - log_softmax on-chip; output [256, 10] per core, concatenated on host.
"""

import numpy as np

IN_CH, HIDDEN, NCLS = 13, 20, 10
B_FULL, NCORE = 2048, 8
B = B_FULL // NCORE          # images per core
NG = 16                      # image groups per core
GI = B // NG                 # images per group (16)
H_GRID = 0.4                 # KAN knot spacing; u = (x + 2.2) / 0.4

CG1 = [(0, 2), (2, 2), (4, 2), (6, 2), (8, 2), (10, 2), (12, 1)]
CG2 = [(0, 4), (4, 4), (8, 4), (12, 1)]
CG3 = [(0, 8), (8, 5)]

_BUILT = None  # cached (nc, input_names)


# ----------------------------------------------------------------------------
# host-side constant packing
# ----------------------------------------------------------------------------

def _pack_bands(w, S, cgs, slotM, rows):
    So = S // 2
    out = np.zeros((rows, len(cgs) * 3 * slotM), np.float32)
    for gi, (c0, nch) in enumerate(cgs):
        for b in range(3):
            col0 = (gi * 3 + b) * slotM
            for cl in range(nch):
                for i in range(So):
                    for a in range(3):
                        r = 2 * i + a - 1
                        if 0 <= r < S:
                            out[cl * S + r, col0 + cl * So + i] = w[c0 + cl, 0, a, b]
    return out


def _pack_bias(bias, cgs, So, rows):
    out = np.zeros((rows, len(cgs)), np.float32)
    for gi, (c0, nch) in enumerate(cgs):
        for cl in range(nch):
            out[cl * So:(cl + 1) * So, gi] = bias[c0 + cl]
    return out


def _host_consts(inp):
    c = {}
    c["bands1"] = _pack_bands(inp["w1"], 64, CG1, 64, 128)
    c["bands2"] = _pack_bands(inp["w2"], 32, CG2, 64, 128)
    c["bands3"] = _pack_bands(inp["w3"], 16, CG3, 64, 128)
    c["bands4"] = _pack_bands(inp["w4"], 8, [(0, 13)], 52, 104)
    c["bands5"] = _pack_bands(inp["w5"], 4, [(0, 13)], 26, 52)
    b6m = np.zeros((26, 26), np.float32)
    for bb in range(2):
        for ch in range(13):
            for r in range(2):
                b6m[ch * 2 + r, bb * 13 + ch] = inp["w6"][ch, 0, r, bb]
    c["bands6"] = b6m
    c["bv1"] = _pack_bias(inp["b1"], CG1, 32, 64)
    c["bv2"] = _pack_bias(inp["b2"], CG2, 16, 64)
    c["bv3"] = _pack_bias(inp["b3"], CG3, 8, 64)
    c["bv4"] = _pack_bias(inp["b4"], [(0, 13)], 4, 52)
    c["bv5"] = _pack_bias(inp["b5"], [(0, 13)], 2, 26)
    c["ubias"] = (2.5 * (inp["b6"] + 2.2)).astype(np.float32).reshape(13, 1)

    # KAN layer 1 rhs pieces: silu part [13,20]; basis+bias part [105,20]
    c["c1a"] = np.ascontiguousarray(inp["sb1"].astype(np.float32))
    c1b = np.zeros((105, HIDDEN), np.float32)
    for n in range(8):
        for i in range(13):
            c1b[n * 13 + i] = inp["coef1"][i, :, n] * inp["ss1"][i] / 6.0
    c1b[104] = inp["bias1"]
    c["c1b"] = c1b
    # KAN layer 2 rhs pieces: silu [20,10]; basis rows 0..107 [108,10]; rows 108..159 + bias [53,10]
    c["c2s"] = np.ascontiguousarray(inp["sb2"].astype(np.float32))
    c2b = np.zeros((161, NCLS), np.float32)
    for n in range(8):
        for i in range(20):
            c2b[n * 20 + i] = inp["coef2"][i, :, n] * inp["ss2"][i] / 6.0
    c2b[160] = inp["bias2"]
    c["c2b1"] = np.ascontiguousarray(c2b[0:108])
    c["c2b2"] = np.ascontiguousarray(c2b[108:161])
    misc = np.zeros((128, 3), np.float32)
    misc[:, 0] = -2.2
    misc[:, 1] = 5.5
    c["misc"] = misc
    c["iden"] = np.eye(128, dtype=np.float32)
    return c


def _shard_x(x_shard):
    # [256,13,64,64] -> xa [16,6,128,16,64] (channel pairs), xb [16,64,16,64] (ch 12)
    xs = x_shard.reshape(NG, GI, 13, 64, 64)
    xa = xs[:, :, 0:12].transpose(0, 2, 3, 1, 4).reshape(NG, 6, 128, GI, 64)
    xb = xs[:, :, 12].transpose(0, 2, 1, 3)
    return np.ascontiguousarray(xa), np.ascontiguousarray(xb)


# ----------------------------------------------------------------------------
# bass program
# ----------------------------------------------------------------------------

def _build():
    global _BUILT
    if _BUILT is not None:
        return _BUILT
    from contextlib import ExitStack
    import concourse.bass as bass  # noqa: F401
    import concourse.bacc as bacc
    import concourse.tile as tile
    import concourse.mybir as mybir

    f32 = mybir.dt.float32
    AF = mybir.ActivationFunctionType
    OP = mybir.AluOpType
    AX = mybir.AxisListType

    nc = bacc.Bacc("TRN2")
    T = nc.tensor

    d_xa = nc.dram_tensor("xa", [NG, 6, 128, GI, 64], f32, kind="ExternalInput")
    d_xb = nc.dram_tensor("xb", [NG, 64, GI, 64], f32, kind="ExternalInput")
    cons_shapes = {
        "bands1": [128, 21 * 64], "bands2": [128, 12 * 64], "bands3": [128, 6 * 64],
        "bands4": [104, 3 * 52], "bands5": [52, 3 * 26], "bands6": [26, 26],
        "bv1": [64, 7], "bv2": [64, 4], "bv3": [64, 2], "bv4": [52, 1],
        "bv5": [26, 1], "ubias": [13, 1], "misc": [128, 3], "c1a": [13, 20],
        "c1b": [105, 20], "c2s": [20, 10], "c2b1": [108, 10], "c2b2": [53, 10],
        "iden": [128, 128],
    }
    d_cons = {k: nc.dram_tensor(k, v, f32, kind="ExternalInput")
              for k, v in cons_shapes.items()}
    d_out = nc.dram_tensor("out", [B, NCLS], f32, kind="ExternalOutput")

    with tile.TileContext(nc) as tc, ExitStack() as ctx:
        cpool = ctx.enter_context(tc.tile_pool(name="consts", bufs=1))
        tcons = {}
        for k, shp in cons_shapes.items():
            t = cpool.tile(shp, f32, name=f"t_{k}")
            nc.sync.dma_start(t[:, :], d_cons[k][:, :])
            tcons[k] = t
        t_u = cpool.tile([13, B], f32, name="t_u")

        p_x1 = ctx.enter_context(tc.tile_pool(name="x1", bufs=3))
        p_x2 = ctx.enter_context(tc.tile_pool(name="x2", bufs=2))
        p_x3 = ctx.enter_context(tc.tile_pool(name="x3", bufs=2))
        p_sm = ctx.enter_context(tc.tile_pool(name="xsm", bufs=2))

        with tc.tile_pool(name="ps1", bufs=2, space="PSUM") as pp1, \
             tc.tile_pool(name="ps2", bufs=2, space="PSUM") as pp2, \
             tc.tile_pool(name="ps3", bufs=1, space="PSUM") as pp3, \
             tc.tile_pool(name="ps456", bufs=2, space="PSUM") as pp456:
            # PE observes each const DMA once, so real matmuls (which allow
            # only a single attached sync-wait on the LDWEIGHTS) never need
            # a const-DMA wait on top of their data wait.
            scrap = pp1.tile([1, 16], f32, tag="scrap", bufs=1, name="scrap")
            for k in ("bands1", "bands2", "bands3", "bands4", "bands5",
                      "bands6", "c1a", "c1b", "c2s", "c2b1", "c2b2", "iden"):
                T.matmul(scrap[0:1, 0:1], tcons[k][0:1, 0:1],
                         tcons[k][0:1, 0:1], start=True, stop=True)
            for g in range(NG):
                # ---- conv1 + scatter into 4ch-blocked x2 tiles ----
                x2t = [p_x2.tile([128, GI * 32], f32, tag=f"x2_{k}", name=f"x2_{k}")
                       for k in range(4)]
                for cg, (c0, nch) in enumerate(CG1):
                    K, M = nch * 64, nch * 32
                    xt = p_x1.tile([128, GI * 64], f32, tag="x1", name="xt")
                    if nch == 2:
                        nc.sync.dma_start(
                            xt[:, :].rearrange("p (i w) -> p i w", w=64),
                            d_xa[g, cg, :, :, :])
                    else:
                        nc.sync.dma_start(
                            xt[0:64, :].rearrange("p (i w) -> p i w", w=64),
                            d_xb[g, :, :, :])
                    # absorb the input-DMA wait into a scrap matmul so the
                    # real matmuls carry only the PSUM-slot (ACT) wait
                    T.matmul(scrap[0:1, 0:1], xt[0:1, 0:1], xt[0:1, 0:1],
                             start=True, stop=True)
                    xv = xt[0:K, :].rearrange("p (i w) -> p i w", w=64)
                    ps = pp1.tile([64, GI * 32], f32, tag="ps1", name="ps1t")
                    pv = ps[0:M, :].rearrange("p (i w) -> p i w", w=32)
                    lo = lambda b: (cg * 3 + b) * 64
                    T.matmul(pv, tcons["bands1"][0:K, lo(1):lo(1) + M],
                             xv[:, :, 0:64:2], start=True, stop=False)
                    T.matmul(pv, tcons["bands1"][0:K, lo(2):lo(2) + M],
                             xv[:, :, 1:64:2], start=False, stop=False)
                    T.matmul(pv[:, :, 1:32], tcons["bands1"][0:K, lo(0):lo(0) + M],
                             xv[:, :, 1:62:2], start=False, stop=True,
                             skip_group_check=True)
                    dest = x2t[cg // 2][64 * (cg % 2):64 * (cg % 2) + M, :]
                    bap = tcons["bv1"][0:M, cg:cg + 1]
                    nc.scalar.activation(dest, ps[0:M, :], AF.Relu, bias=bap)

                # ---- conv2 -> x3 tiles (8ch-blocked) ----
                x3t = [p_x3.tile([128, GI * 16], f32, tag=f"x3_{k}", name=f"x3_{k}")
                       for k in range(2)]
                for k4, (c0, nch) in enumerate(CG2):
                    K, M = nch * 32, nch * 16
                    xv = x2t[k4][0:K, :].rearrange("p (i w) -> p i w", w=32)
                    ps = pp2.tile([64, GI * 16], f32, tag="ps2", name="ps2t")
                    pv = ps[0:M, :].rearrange("p (i w) -> p i w", w=16)
                    lo = lambda b: (k4 * 3 + b) * 64
                    T.matmul(pv, tcons["bands2"][0:K, lo(1):lo(1) + M],
                             xv[:, :, 0:32:2], start=True, stop=False)
                    T.matmul(pv, tcons["bands2"][0:K, lo(2):lo(2) + M],
                             xv[:, :, 1:32:2], start=False, stop=False)
                    T.matmul(pv[:, :, 1:16], tcons["bands2"][0:K, lo(0):lo(0) + M],
                             xv[:, :, 1:30:2], start=False, stop=True,
                             skip_group_check=True)
                    dest = x3t[k4 // 2][64 * (k4 % 2):64 * (k4 % 2) + M, :]
                    bap = tcons["bv2"][0:M, k4:k4 + 1]
                    nc.scalar.activation(dest, ps[0:M, :], AF.Relu, bias=bap)

                # ---- conv3 -> x4 [104, GI*8] ----
                x4 = p_sm.tile([104, GI * 8], f32, tag="x4", name="x4")
                for k8, (c0, nch) in enumerate(CG3):
                    K, M = nch * 16, nch * 8
                    xv = x3t[k8][0:K, :].rearrange("p (i w) -> p i w", w=16)
                    ps = pp3.tile([64, GI * 8], f32, tag="ps3", name="ps3t")
                    pv = ps[0:M, :].rearrange("p (i w) -> p i w", w=8)
                    lo = lambda b: (k8 * 3 + b) * 64
                    T.matmul(pv, tcons["bands3"][0:K, lo(1):lo(1) + M],
                             xv[:, :, 0:16:2], start=True, stop=False)
                    T.matmul(pv, tcons["bands3"][0:K, lo(2):lo(2) + M],
                             xv[:, :, 1:16:2], start=False, stop=False)
                    T.matmul(pv[:, :, 1:8], tcons["bands3"][0:K, lo(0):lo(0) + M],
                             xv[:, :, 1:14:2], start=False, stop=True,
                             skip_group_check=True)
                    dest = x4[64 * k8:64 * k8 + M, :]
                    bap = tcons["bv3"][0:M, k8:k8 + 1]
                    nc.scalar.activation(dest, ps[0:M, :], AF.Relu, bias=bap)

                # ---- conv4 [104 -> 52] ----
                x5 = p_sm.tile([52, GI * 4], f32, tag="x5", name="x5")
                xv = x4[0:104, :].rearrange("p (i w) -> p i w", w=8)
                ps4 = pp456.tile([64, GI * 4], f32, tag="ps456", name="ps4t")
                pv = ps4[0:52, :].rearrange("p (i w) -> p i w", w=4)
                T.matmul(pv, tcons["bands4"][0:104, 52:104], xv[:, :, 0:8:2],
                         start=True, stop=False)
                T.matmul(pv, tcons["bands4"][0:104, 104:156], xv[:, :, 1:8:2],
                         start=False, stop=False)
                T.matmul(pv[:, :, 1:4], tcons["bands4"][0:104, 0:52],
                         xv[:, :, 1:6:2], start=False, stop=True,
                         skip_group_check=True)
                nc.scalar.activation(x5[:, :], ps4[0:52, :], AF.Relu,
                                     bias=tcons["bv4"][0:52, 0:1])

                # ---- conv5 [52 -> 26] ----
                x6 = p_sm.tile([26, GI * 2], f32, tag="x6", name="x6")
                xv = x5[0:52, :].rearrange("p (i w) -> p i w", w=4)
                ps5 = pp456.tile([64, GI * 2], f32, tag="ps456", name="ps5t")
                pv = ps5[0:26, :].rearrange("p (i w) -> p i w", w=2)
                T.matmul(pv, tcons["bands5"][0:52, 26:52], xv[:, :, 0:4:2],
                         start=True, stop=False)
                T.matmul(pv, tcons["bands5"][0:52, 52:78], xv[:, :, 1:4:2],
                         start=False, stop=False)
                T.matmul(pv[:, :, 1:2], tcons["bands5"][0:52, 0:26],
                         xv[:, :, 1:2:2], start=False, stop=True,
                         skip_group_check=True)
                nc.scalar.activation(x6[:, :], ps5[0:26, :], AF.Relu,
                                     bias=tcons["bv5"][0:26, 0:1])

                # ---- conv6 (2x2 valid) -> u[:, g*GI : g*GI+GI] ----
                xv = x6[0:26, :].rearrange("p (i w) -> p i w", w=2)
                ps6 = pp456.tile([64, GI], f32, tag="ps456", name="ps6t")
                T.matmul(ps6[0:13, :], tcons["bands6"][0:26, 0:13],
                         xv[:, :, 0:1], start=True, stop=False)
                T.matmul(ps6[0:13, :], tcons["bands6"][0:26, 13:26],
                         xv[:, :, 1:2], start=False, stop=True)
                nc.scalar.activation(t_u[:, g * GI:(g + 1) * GI], ps6[0:13, :],
                                     AF.Identity, bias=tcons["ubias"][0:13, 0:1],
                                     scale=2.5)

        # ------------------------------ KAN head ------------------------------
        kpool = ctx.enter_context(tc.tile_pool(name="kan", bufs=2))
        with tc.tile_pool(name="psk", bufs=1, space="PSUM") as ppk:
            for t in range(B // 128):
                sl = slice(t * 128, (t + 1) * 128)
                # uT [128, 13] via PE transpose
                ps_uT = ppk.tile([128, 128], f32, tag="uT", name="ps_uT")
                T.transpose(ps_uT[0:128, 0:13], t_u[0:13, sl], tcons["iden"][0:13, 0:13])
                # D[:, k*13 : k*13+13] = u - k
                D = kpool.tile([128, 156], f32, tag="D", name="Dt")
                for k in range(12):
                    nc.vector.tensor_scalar(D[:, k * 13:(k + 1) * 13],
                                            ps_uT[0:128, 0:13], float(-k), None,
                                            op0=OP.add)
                # degree-0 basis
                ge = kpool.tile([128, 143], f32, tag="ge", name="ge")
                lt = kpool.tile([128, 143], f32, tag="lt", name="lt")
                nc.vector.tensor_scalar(ge[:, :], D[:, 0:143], 0.0, None, op0=OP.is_ge)
                nc.vector.tensor_scalar(lt[:, :], D[:, 13:156], 0.0, None, op0=OP.is_lt)
                Bc = kpool.tile([128, 143], f32, tag="B0", name="Bc")
                nc.vector.tensor_mul(Bc[:, :], ge[:, :], lt[:, :])
                # Cox-de-Boor levels (unnormalized; /6 folded into c1b)
                wid = 143
                for p in range(1, 4):
                    wid -= 13
                    ta = kpool.tile([128, wid], f32, tag=f"ta{p}", name="ta")
                    tb = kpool.tile([128, wid], f32, tag=f"tb{p}", name="tb")
                    nc.vector.tensor_mul(ta[:, :], D[:, 0:wid], Bc[:, 0:wid])
                    nc.vector.tensor_mul(tb[:, :], D[:, 13 * (p + 1):13 * (p + 1) + wid],
                                         Bc[:, 13:13 + wid])
                    if p < 3:
                        Bc = kpool.tile([128, wid], f32, tag=f"B{p}", name="Bc")
                        nc.vector.tensor_sub(Bc[:, :], ta[:, :], tb[:, :])
                    else:
                        Bc = kpool.tile([128, 105], f32, tag="B3", name="Bc")
                        nc.vector.tensor_sub(Bc[:, 0:104], ta[:, :], tb[:, :])
                        nc.vector.memset(Bc[:, 104:105], 1.0)
                # stacks: silu part [13,128]; (basis;1)^T part [105,128]
                stkA = kpool.tile([13, 128], f32, tag="stkA", name="stkA")
                stkB = kpool.tile([105, 128], f32, tag="stkB", name="stkB")
                ps_b1 = ppk.tile([128, 128], f32, tag="b1", name="ps_b1")
                T.transpose(ps_b1[0:105, 0:128], Bc[:, 0:105], tcons["iden"][:, :])
                nc.vector.tensor_copy(stkB[:, :], ps_b1[0:105, 0:128])
                nc.scalar.activation(stkA[:, :], t_u[0:13, sl], AF.Silu,
                                     bias=tcons["misc"][0:13, 0:1], scale=H_GRID)
                ps_h1 = ppk.tile([128, 128], f32, tag="h1", name="ps_h1")
                T.matmul(ps_h1[0:128, 0:20], stkA[:, :], tcons["c1a"][:, :],
                         start=True, stop=False)
                T.matmul(ps_h1[0:128, 0:20], stkB[:, :], tcons["c1b"][:, :],
                         start=False, stop=True)
                # ---- KAN layer 2 ----
                u2 = kpool.tile([128, 20], f32, tag="u2", name="u2")
                nc.scalar.activation(u2[:, :], ps_h1[0:128, 0:20], AF.Identity,
                                     bias=tcons["misc"][0:128, 1:2], scale=2.5)
                stk2s = kpool.tile([20, 128], f32, tag="s2s", name="stk2s")
                ps_t2 = ppk.tile([128, 128], f32, tag="t2", name="ps_t2")
                T.transpose(ps_t2[0:20, 0:128], u2[:, :], tcons["iden"][:, :])
                nc.scalar.activation(stk2s[:, :], ps_t2[0:20, 0:128], AF.Silu,
                                     bias=tcons["misc"][0:20, 0:1], scale=H_GRID)
                D2 = kpool.tile([128, 240], f32, tag="D2", name="D2t")
                for k in range(12):
                    nc.vector.tensor_scalar(D2[:, k * 20:(k + 1) * 20], u2[:, :],
                                            float(-k), None, op0=OP.add)
                ge2 = kpool.tile([128, 220], f32, tag="ge2", name="ge2")
                lt2 = kpool.tile([128, 220], f32, tag="lt2", name="lt2")
                nc.vector.tensor_scalar(ge2[:, :], D2[:, 0:220], 0.0, None, op0=OP.is_ge)
                nc.vector.tensor_scalar(lt2[:, :], D2[:, 20:240], 0.0, None, op0=OP.is_lt)
                Bc2 = kpool.tile([128, 220], f32, tag="B0_2", name="Bc2")
                nc.vector.tensor_mul(Bc2[:, :], ge2[:, :], lt2[:, :])
                wid = 220
                for p in range(1, 4):
                    wid -= 20
                    ta = kpool.tile([128, wid], f32, tag=f"t2a{p}", name="ta2")
                    tb = kpool.tile([128, wid], f32, tag=f"t2b{p}", name="tb2")
                    nc.vector.tensor_mul(ta[:, :], D2[:, 0:wid], Bc2[:, 0:wid])
                    nc.vector.tensor_mul(tb[:, :], D2[:, 20 * (p + 1):20 * (p + 1) + wid],
                                         Bc2[:, 20:20 + wid])
                    if p < 3:
                        Bc2 = kpool.tile([128, wid], f32, tag=f"B{p}_2", name="Bc2")
                        nc.vector.tensor_sub(Bc2[:, :], ta[:, :], tb[:, :])
                    else:
                        Bc2 = kpool.tile([128, 161], f32, tag="B3_2", name="Bc2")
                        nc.vector.tensor_sub(Bc2[:, 0:160], ta[:, :], tb[:, :])
                        nc.vector.memset(Bc2[:, 160:161], 1.0)
                stk2a = kpool.tile([108, 128], f32, tag="s2a", name="stk2a")
                stk2b = kpool.tile([53, 128], f32, tag="s2b", name="stk2b")
                ps_b2 = ppk.tile([128, 128], f32, tag="b2", name="ps_b2")
                T.transpose(ps_b2[0:108, 0:128], Bc2[:, 0:108], tcons["iden"][:, :])
                nc.vector.tensor_copy(stk2a[:, :], ps_b2[0:108, 0:128])
                ps_b3 = ppk.tile([128, 128], f32, tag="b3", name="ps_b3")
                T.transpose(ps_b3[0:53, 0:128], Bc2[:, 108:161], tcons["iden"][:, :])
                nc.vector.tensor_copy(stk2b[:, :], ps_b3[0:53, 0:128])
                ps_lg = ppk.tile([128, 128], f32, tag="lg", name="ps_lg")
                T.matmul(ps_lg[0:128, 0:NCLS], stk2a[:, :], tcons["c2b1"][:, :],
                         start=True, stop=False)
                T.matmul(ps_lg[0:128, 0:NCLS], stk2s[:, :], tcons["c2s"][:, :],
                         start=False, stop=False)
                T.matmul(ps_lg[0:128, 0:NCLS], stk2b[:, :], tcons["c2b2"][:, :],
                         start=False, stop=True)
                # ---- log_softmax (on an SBUF copy; ps_lg has 1 PSUM reader) ----
                lg_s = kpool.tile([128, NCLS], f32, tag="lg_s", name="lg_s")
                nc.vector.tensor_copy(lg_s[:, :], ps_lg[0:128, 0:NCLS])
                negm = kpool.tile([128, 1], f32, tag="negm", name="negm")
                nc.vector.reduce_max(negm[:, :], lg_s[:, :], axis=AX.X,
                                     negate=True)
                ex = kpool.tile([128, NCLS], f32, tag="ex", name="ex")
                nc.scalar.activation(ex[:, :], lg_s[:, :], AF.Exp,
                                     bias=negm[:, 0:1])
                ssum = kpool.tile([128, 1], f32, tag="ssum", name="ssum")
                nc.vector.reduce_sum(ssum[:, :], ex[:, :], axis=AX.X)
                lsum = kpool.tile([128, 1], f32, tag="lsum", name="lsum")
                nc.scalar.activation(lsum[:, :], ssum[:, :], AF.Ln,
                                     bias=tcons["misc"][0:128, 2:3])
                res = kpool.tile([128, NCLS], f32, tag="res", name="res")
                nc.vector.tensor_scalar(res[:, :], lg_s[:, :],
                                        negm[:, 0:1], lsum[:, 0:1],
                                        op0=OP.add, op1=OP.subtract)
                nc.sync.dma_start(d_out[sl, :], res[:, :])

    nc.compile()  # bacc lowering: wait splitting via event semaphores, etc.
    _BUILT = (nc, ["xa", "xb"] + list(cons_shapes.keys()))
    return _BUILT


# ----------------------------------------------------------------------------
# entry point
# ----------------------------------------------------------------------------

def kernel(**inputs):
    from concourse import bass_utils

    x = np.asarray(inputs["x"], np.float32)
    cons = _host_consts({k: np.asarray(v, np.float32)
                         for k, v in inputs.items() if k != "x"})
    nc, _names = _build()

    in_maps = []
    for core in range(NCORE):
        xa, xb = _shard_x(x[core * B:(core + 1) * B])
        in_maps.append({"xa": xa, "xb": xb, **cons})
    res = bass_utils.run_bass_kernel_spmd(nc, in_maps, core_ids=list(range(NCORE)))
    return np.concatenate([r["out"] for r in res.results], axis=0)



# revision 6
# speedup vs baseline: 2.0213x; 2.0213x over previous
"""Trainium2 Bass kernel for nn_ChannelWiseConv (depthwise conv stack + KAN head).

Strategy (per core, pure data parallelism over batch):
  - 256 images/core. Each stride-2 depthwise 3x3 conv is computed as 3 PE
    matmuls accumulating in PSUM: contraction over input rows with per-channel
    banded weight matrices (one per kernel-column tap); the column subsampling
    is expressed in the moving-operand access pattern. Channels are blocked
    (2/4/8/13 per matmul as planes shrink) to keep K near 128.
  - bias+relu fused into one ACT/DVE op (PSUM -> SBUF) that also scatters into
    the next layer's channel-blocked input layout.
  - KAN layers: B-spline basis via unnormalized Cox-de-Boor on a uniform grid
    (all denominators folded into host-precomputed coefficient matrices), then
    one PE matmul per KAN layer over [silu(x); basis; 1] stacks.
  - log_softmax on-chip; output [256, 10] per core, concatenated on host.
"""

import ml_dtypes
import numpy as np

BF16 = ml_dtypes.bfloat16
IN_CH, HIDDEN, NCLS = 13, 20, 10
B_FULL, NCORE = 2048, 8
B = B_FULL // NCORE          # images per core
NG = 16                      # image groups per core
GI = B // NG                 # images per group (16)
H_GRID = 0.4                 # KAN knot spacing; u = (x + 2.2) / 0.4

CG1 = [(0, 2), (2, 2), (4, 2), (6, 2), (8, 2), (10, 2), (12, 1)]
CG2 = [(0, 4), (4, 4), (8, 4), (12, 1)]
CG3 = [(0, 8), (8, 5)]

_BUILT = None  # cached (nc, input_names)


# ----------------------------------------------------------------------------
# host-side constant packing
# ----------------------------------------------------------------------------

def _pack_bands(w, S, cgs, slotM, rows):
    So = S // 2
    out = np.zeros((rows, len(cgs) * 3 * slotM), np.float32)
    for gi, (c0, nch) in enumerate(cgs):
        for b in range(3):
            col0 = (gi * 3 + b) * slotM
            for cl in range(nch):
                for i in range(So):
                    for a in range(3):
                        r = 2 * i + a - 1
                        if 0 <= r < S:
                            out[cl * S + r, col0 + cl * So + i] = w[c0 + cl, 0, a, b]
    return out


def _pack_bias(bias, cgs, So, rows):
    out = np.zeros((rows, len(cgs)), np.float32)
    for gi, (c0, nch) in enumerate(cgs):
        for cl in range(nch):
            out[cl * So:(cl + 1) * So, gi] = bias[c0 + cl]
    return out


def _host_consts(inp):
    c = {}
    c["bands1"] = _pack_bands(inp["w1"], 64, CG1, 64, 128).astype(BF16)
    c["bands2"] = _pack_bands(inp["w2"], 32, CG2, 64, 128).astype(BF16)
    c["bands3"] = _pack_bands(inp["w3"], 16, CG3, 64, 128).astype(BF16)
    c["bands4"] = _pack_bands(inp["w4"], 8, [(0, 13)], 52, 104).astype(BF16)
    c["bands5"] = _pack_bands(inp["w5"], 4, [(0, 13)], 26, 52).astype(BF16)
    b6m = np.zeros((26, 26), np.float32)
    for bb in range(2):
        for ch in range(13):
            for r in range(2):
                b6m[ch * 2 + r, bb * 13 + ch] = inp["w6"][ch, 0, r, bb]
    c["bands6"] = b6m.astype(BF16)
    c["bv1"] = _pack_bias(inp["b1"], CG1, 32, 64)
    c["bv2"] = _pack_bias(inp["b2"], CG2, 16, 64)
    c["bv3"] = _pack_bias(inp["b3"], CG3, 8, 64)
    c["bv4"] = _pack_bias(inp["b4"], [(0, 13)], 4, 52)
    c["bv5"] = _pack_bias(inp["b5"], [(0, 13)], 2, 26)
    c["ubias"] = (2.5 * (inp["b6"] + 2.2)).astype(np.float32).reshape(13, 1)

    # KAN layer 1 rhs pieces: silu part [13,20]; basis+bias part [105,20]
    c["c1a"] = np.ascontiguousarray(inp["sb1"].astype(np.float32))
    c1b = np.zeros((105, HIDDEN), np.float32)
    for n in range(8):
        for i in range(13):
            c1b[n * 13 + i] = inp["coef1"][i, :, n] * inp["ss1"][i] / 6.0
    c1b[104] = inp["bias1"]
    c["c1b"] = c1b
    # KAN layer 2 rhs pieces: silu [20,10]; basis rows 0..107 [108,10]; rows 108..159 + bias [53,10]
    c["c2s"] = np.ascontiguousarray(inp["sb2"].astype(np.float32))
    c2b = np.zeros((161, NCLS), np.float32)
    for n in range(8):
        for i in range(20):
            c2b[n * 20 + i] = inp["coef2"][i, :, n] * inp["ss2"][i] / 6.0
    c2b[160] = inp["bias2"]
    c["c2b1"] = np.ascontiguousarray(c2b[0:108])
    c["c2b2"] = np.ascontiguousarray(c2b[108:161])
    misc = np.zeros((128, 3), np.float32)
    misc[:, 0] = -2.2
    misc[:, 1] = 5.5
    c["misc"] = misc
    c["iden"] = np.eye(128, dtype=np.float32)
    return c


def _shard_x(x_shard):
    # [256,13,64,64] -> xa [16,6,128,16,64] (channel pairs), xb [16,64,16,64] (ch 12)
    xs = x_shard.reshape(NG, GI, 13, 64, 64)
    xa = xs[:, :, 0:12].transpose(0, 2, 3, 1, 4).reshape(NG, 6, 128, GI, 64)
    xb = xs[:, :, 12].transpose(0, 2, 1, 3)
    return xa.astype(BF16), xb.astype(BF16)


# ----------------------------------------------------------------------------
# bass program
# ----------------------------------------------------------------------------

def _build():
    global _BUILT
    if _BUILT is not None:
        return _BUILT
    from contextlib import ExitStack
    import concourse.bass as bass  # noqa: F401
    import concourse.bacc as bacc
    import concourse.tile as tile
    import concourse.mybir as mybir

    f32 = mybir.dt.float32
    bf16 = mybir.dt.bfloat16
    AF = mybir.ActivationFunctionType
    OP = mybir.AluOpType
    AX = mybir.AxisListType

    nc = bacc.Bacc("TRN2")
    T = nc.tensor

    d_xa = nc.dram_tensor("xa", [NG, 6, 128, GI, 64], bf16, kind="ExternalInput")
    d_xb = nc.dram_tensor("xb", [NG, 64, GI, 64], bf16, kind="ExternalInput")
    cons_shapes = {
        "bands1": [128, 21 * 64], "bands2": [128, 12 * 64], "bands3": [128, 6 * 64],
        "bands4": [104, 3 * 52], "bands5": [52, 3 * 26], "bands6": [26, 26],
        "bv1": [64, 7], "bv2": [64, 4], "bv3": [64, 2], "bv4": [52, 1],
        "bv5": [26, 1], "ubias": [13, 1], "misc": [128, 3], "c1a": [13, 20],
        "c1b": [105, 20], "c2s": [20, 10], "c2b1": [108, 10], "c2b2": [53, 10],
        "iden": [128, 128],
    }
    BANDS = {"bands1", "bands2", "bands3", "bands4", "bands5", "bands6"}
    d_cons = {k: nc.dram_tensor(k, v, bf16 if k in BANDS else f32,
                                kind="ExternalInput")
              for k, v in cons_shapes.items()}
    d_out = nc.dram_tensor("out", [B, NCLS], f32, kind="ExternalOutput")

    with tile.TileContext(nc) as tc, ExitStack() as ctx:
        cpool = ctx.enter_context(tc.tile_pool(name="consts", bufs=1))
        tcons = {}
        for k, shp in cons_shapes.items():
            t = cpool.tile(shp, bf16 if k in BANDS else f32, name=f"t_{k}")
            nc.sync.dma_start(t[:, :], d_cons[k][:, :])
            tcons[k] = t
        t_u = cpool.tile([13, B], f32, name="t_u")

        p_x1 = ctx.enter_context(tc.tile_pool(name="x1", bufs=3))
        p_x2 = ctx.enter_context(tc.tile_pool(name="x2", bufs=2))
        p_x3 = ctx.enter_context(tc.tile_pool(name="x3", bufs=2))
        p_sm = ctx.enter_context(tc.tile_pool(name="xsm", bufs=2))

        with tc.tile_pool(name="ps1", bufs=2, space="PSUM") as pp1, \
             tc.tile_pool(name="ps2", bufs=2, space="PSUM") as pp2, \
             tc.tile_pool(name="ps3", bufs=1, space="PSUM") as pp3, \
             tc.tile_pool(name="ps456", bufs=2, space="PSUM") as pp456:
            # PE observes each const DMA once, so real matmuls (which allow
            # only a single attached sync-wait on the LDWEIGHTS) never need
            # a const-DMA wait on top of their data wait.
            scrap = pp1.tile([1, 16], f32, tag="scrap", bufs=1, name="scrap")
            for k in ("bands1", "bands2", "bands3", "bands4", "bands5",
                      "bands6", "c1a", "c1b", "c2s", "c2b1", "c2b2", "iden"):
                T.matmul(scrap[0:1, 0:1], tcons[k][0:1, 0:1],
                         tcons[k][0:1, 0:1], start=True, stop=True)
            for g in range(NG):
                # ---- conv1 + scatter into 4ch-blocked x2 tiles ----
                x2t = [p_x2.tile([128, GI * 32], bf16, tag=f"x2_{k}", name=f"x2_{k}")
                       for k in range(4)]
                for cg, (c0, nch) in enumerate(CG1):
                    K, M = nch * 64, nch * 32
                    xt = p_x1.tile([128, GI * 64], bf16, tag="x1", name="xt")
                    if nch == 2:
                        nc.sync.dma_start(
                            xt[:, :].rearrange("p (i w) -> p i w", w=64),
                            d_xa[g, cg, :, :, :])
                    else:
                        nc.sync.dma_start(
                            xt[0:64, :].rearrange("p (i w) -> p i w", w=64),
                            d_xb[g, :, :, :])
                    # absorb the input-DMA wait into a scrap matmul so the
                    # real matmuls carry only the PSUM-slot (ACT) wait
                    T.matmul(scrap[0:1, 0:1], xt[0:1, 0:1], xt[0:1, 0:1],
                             start=True, stop=True)
                    xv = xt[0:K, :].rearrange("p (i w) -> p i w", w=64)
                    ps = pp1.tile([64, GI * 32], f32, tag="ps1", name="ps1t")
                    pv = ps[0:M, :].rearrange("p (i w) -> p i w", w=32)
                    lo = lambda b: (cg * 3 + b) * 64
                    T.matmul(pv, tcons["bands1"][0:K, lo(1):lo(1) + M],
                             xv[:, :, 0:64:2], start=True, stop=False)
                    T.matmul(pv, tcons["bands1"][0:K, lo(2):lo(2) + M],
                             xv[:, :, 1:64:2], start=False, stop=False)
                    T.matmul(pv[:, :, 1:32], tcons["bands1"][0:K, lo(0):lo(0) + M],
                             xv[:, :, 1:62:2], start=False, stop=True,
                             skip_group_check=True)
                    dest = x2t[cg // 2][64 * (cg % 2):64 * (cg % 2) + M, :]
                    bap = tcons["bv1"][0:M, cg:cg + 1]
                    nc.scalar.activation(dest, ps[0:M, :], AF.Relu, bias=bap)

                # ---- conv2 -> x3 tiles (8ch-blocked) ----
                x3t = [p_x3.tile([128, GI * 16], bf16, tag=f"x3_{k}", name=f"x3_{k}")
                       for k in range(2)]
                for k4, (c0, nch) in enumerate(CG2):
                    K, M = nch * 32, nch * 16
                    xv = x2t[k4][0:K, :].rearrange("p (i w) -> p i w", w=32)
                    ps = pp2.tile([64, GI * 16], f32, tag="ps2", name="ps2t")
                    pv = ps[0:M, :].rearrange("p (i w) -> p i w", w=16)
                    lo = lambda b: (k4 * 3 + b) * 64
                    T.matmul(pv, tcons["bands2"][0:K, lo(1):lo(1) + M],
                             xv[:, :, 0:32:2], start=True, stop=False)
                    T.matmul(pv, tcons["bands2"][0:K, lo(2):lo(2) + M],
                             xv[:, :, 1:32:2], start=False, stop=False)
                    T.matmul(pv[:, :, 1:16], tcons["bands2"][0:K, lo(0):lo(0) + M],
                             xv[:, :, 1:30:2], start=False, stop=True,
                             skip_group_check=True)
                    dest = x3t[k4 // 2][64 * (k4 % 2):64 * (k4 % 2) + M, :]
                    bap = tcons["bv2"][0:M, k4:k4 + 1]
                    nc.scalar.activation(dest, ps[0:M, :], AF.Relu, bias=bap)

                # ---- conv3 -> x4 [104, GI*8] ----
                x4 = p_sm.tile([104, GI * 8], bf16, tag="x4", name="x4")
                for k8, (c0, nch) in enumerate(CG3):
                    K, M = nch * 16, nch * 8
                    xv = x3t[k8][0:K, :].rearrange("p (i w) -> p i w", w=16)
                    ps = pp3.tile([64, GI * 8], f32, tag="ps3", name="ps3t")
                    pv = ps[0:M, :].rearrange("p (i w) -> p i w", w=8)
                    lo = lambda b: (k8 * 3 + b) * 64
                    T.matmul(pv, tcons["bands3"][0:K, lo(1):lo(1) + M],
                             xv[:, :, 0:16:2], start=True, stop=False)
                    T.matmul(pv, tcons["bands3"][0:K, lo(2):lo(2) + M],
                             xv[:, :, 1:16:2], start=False, stop=False)
                    T.matmul(pv[:, :, 1:8], tcons["bands3"][0:K, lo(0):lo(0) + M],
                             xv[:, :, 1:14:2], start=False, stop=True,
                             skip_group_check=True)
                    dest = x4[64 * k8:64 * k8 + M, :]
                    bap = tcons["bv3"][0:M, k8:k8 + 1]
                    nc.scalar.activation(dest, ps[0:M, :], AF.Relu, bias=bap)

                # ---- conv4 [104 -> 52] ----
                x5 = p_sm.tile([52, GI * 4], bf16, tag="x5", name="x5")
                xv = x4[0:104, :].rearrange("p (i w) -> p i w", w=8)
                ps4 = pp456.tile([64, GI * 4], f32, tag="ps456", name="ps4t")
                pv = ps4[0:52, :].rearrange("p (i w) -> p i w", w=4)
                T.matmul(pv, tcons["bands4"][0:104, 52:104], xv[:, :, 0:8:2],
                         start=True, stop=False)
                T.matmul(pv, tcons["bands4"][0:104, 104:156], xv[:, :, 1:8:2],
                         start=False, stop=False)
                T.matmul(pv[:, :, 1:4], tcons["bands4"][0:104, 0:52],
                         xv[:, :, 1:6:2], start=False, stop=True,
                         skip_group_check=True)
                nc.scalar.activation(x5[:, :], ps4[0:52, :], AF.Relu,
                                     bias=tcons["bv4"][0:52, 0:1])

                # ---- conv5 [52 -> 26] ----
                x6 = p_sm.tile([26, GI * 2], bf16, tag="x6", name="x6")
                xv = x5[0:52, :].rearrange("p (i w) -> p i w", w=4)
                ps5 = pp456.tile([64, GI * 2], f32, tag="ps456", name="ps5t")
                pv = ps5[0:26, :].rearrange("p (i w) -> p i w", w=2)
                T.matmul(pv, tcons["bands5"][0:52, 26:52], xv[:, :, 0:4:2],
                         start=True, stop=False)
                T.matmul(pv, tcons["bands5"][0:52, 52:78], xv[:, :, 1:4:2],
                         start=False, stop=False)
                T.matmul(pv[:, :, 1:2], tcons["bands5"][0:52, 0:26],
                         xv[:, :, 1:2:2], start=False, stop=True,
                         skip_group_check=True)
                nc.scalar.activation(x6[:, :], ps5[0:26, :], AF.Relu,
                                     bias=tcons["bv5"][0:26, 0:1])

                # ---- conv6 (2x2 valid) -> u[:, g*GI : g*GI+GI] ----
                xv = x6[0:26, :].rearrange("p (i w) -> p i w", w=2)
                ps6 = pp456.tile([64, GI], f32, tag="ps456", name="ps6t")
                T.matmul(ps6[0:13, :], tcons["bands6"][0:26, 0:13],
                         xv[:, :, 0:1], start=True, stop=False)
                T.matmul(ps6[0:13, :], tcons["bands6"][0:26, 13:26],
                         xv[:, :, 1:2], start=False, stop=True)
                nc.scalar.activation(t_u[:, g * GI:(g + 1) * GI], ps6[0:13, :],
                                     AF.Identity, bias=tcons["ubias"][0:13, 0:1],
                                     scale=2.5)

        # ------------------------------ KAN head ------------------------------
        kpool = ctx.enter_context(tc.tile_pool(name="kan", bufs=2))
        with tc.tile_pool(name="psk", bufs=1, space="PSUM") as ppk:
            for t in range(B // 128):
                sl = slice(t * 128, (t + 1) * 128)
                # uT [128, 13] via PE transpose
                ps_uT = ppk.tile([128, 128], f32, tag="uT", name="ps_uT")
                T.transpose(ps_uT[0:128, 0:13], t_u[0:13, sl], tcons["iden"][0:13, 0:13])
                # D[:, k*13 : k*13+13] = u - k
                D = kpool.tile([128, 156], f32, tag="D", name="Dt")
                for k in range(12):
                    nc.vector.tensor_scalar(D[:, k * 13:(k + 1) * 13],
                                            ps_uT[0:128, 0:13], float(-k), None,
                                            op0=OP.add)
                # degree-0 basis
                ge = kpool.tile([128, 143], f32, tag="ge", name="ge")
                lt = kpool.tile([128, 143], f32, tag="lt", name="lt")
                nc.vector.tensor_scalar(ge[:, :], D[:, 0:143], 0.0, None, op0=OP.is_ge)
                nc.vector.tensor_scalar(lt[:, :], D[:, 13:156], 0.0, None, op0=OP.is_lt)
                Bc = kpool.tile([128, 143], f32, tag="B0", name="Bc")
                nc.vector.tensor_mul(Bc[:, :], ge[:, :], lt[:, :])
                # Cox-de-Boor levels (unnormalized; /6 folded into c1b)
                wid = 143
                for p in range(1, 4):
                    wid -= 13
                    ta = kpool.tile([128, wid], f32, tag=f"ta{p}", name="ta")
                    tb = kpool.tile([128, wid], f32, tag=f"tb{p}", name="tb")
                    nc.vector.tensor_mul(ta[:, :], D[:, 0:wid], Bc[:, 0:wid])
                    nc.vector.tensor_mul(tb[:, :], D[:, 13 * (p + 1):13 * (p + 1) + wid],
                                         Bc[:, 13:13 + wid])
                    if p < 3:
                        Bc = kpool.tile([128, wid], f32, tag=f"B{p}", name="Bc")
                        nc.vector.tensor_sub(Bc[:, :], ta[:, :], tb[:, :])
                    else:
                        Bc = kpool.tile([128, 105], f32, tag="B3", name="Bc")
                        nc.vector.tensor_sub(Bc[:, 0:104], ta[:, :], tb[:, :])
                        nc.vector.memset(Bc[:, 104:105], 1.0)
                # stacks: silu part [13,128]; (basis;1)^T part [105,128]
                stkA = kpool.tile([13, 128], f32, tag="stkA", name="stkA")
                stkB = kpool.tile([105, 128], f32, tag="stkB", name="stkB")
                ps_b1 = ppk.tile([128, 128], f32, tag="b1", name="ps_b1")
                T.transpose(ps_b1[0:105, 0:128], Bc[:, 0:105], tcons["iden"][:, :])
                nc.vector.tensor_copy(stkB[:, :], ps_b1[0:105, 0:128])
                nc.scalar.activation(stkA[:, :], t_u[0:13, sl], AF.Silu,
                                     bias=tcons["misc"][0:13, 0:1], scale=H_GRID)
                ps_h1 = ppk.tile([128, 128], f32, tag="h1", name="ps_h1")
                T.matmul(ps_h1[0:128, 0:20], stkA[:, :], tcons["c1a"][:, :],
                         start=True, stop=False)
                T.matmul(ps_h1[0:128, 0:20], stkB[:, :], tcons["c1b"][:, :],
                         start=False, stop=True)
                # ---- KAN layer 2 ----
                u2 = kpool.tile([128, 20], f32, tag="u2", name="u2")
                nc.scalar.activation(u2[:, :], ps_h1[0:128, 0:20], AF.Identity,
                                     bias=tcons["misc"][0:128, 1:2], scale=2.5)
                stk2s = kpool.tile([20, 128], f32, tag="s2s", name="stk2s")
                ps_t2 = ppk.tile([128, 128], f32, tag="t2", name="ps_t2")
                T.transpose(ps_t2[0:20, 0:128], u2[:, :], tcons["iden"][:, :])
                nc.scalar.activation(stk2s[:, :], ps_t2[0:20, 0:128], AF.Silu,
                                     bias=tcons["misc"][0:20, 0:1], scale=H_GRID)
                D2 = kpool.tile([128, 240], f32, tag="D2", name="D2t")
                for k in range(12):
                    nc.vector.tensor_scalar(D2[:, k * 20:(k + 1) * 20], u2[:, :],
                                            float(-k), None, op0=OP.add)
                ge2 = kpool.tile([128, 220], f32, tag="ge2", name="ge2")
                lt2 = kpool.tile([128, 220], f32, tag="lt2", name="lt2")
                nc.vector.tensor_scalar(ge2[:, :], D2[:, 0:220], 0.0, None, op0=OP.is_ge)
                nc.vector.tensor_scalar(lt2[:, :], D2[:, 20:240], 0.0, None, op0=OP.is_lt)
                Bc2 = kpool.tile([128, 220], f32, tag="B0_2", name="Bc2")
                nc.vector.tensor_mul(Bc2[:, :], ge2[:, :], lt2[:, :])
                wid = 220
                for p in range(1, 4):
                    wid -= 20
                    ta = kpool.tile([128, wid], f32, tag=f"t2a{p}", name="ta2")
                    tb = kpool.tile([128, wid], f32, tag=f"t2b{p}", name="tb2")
                    nc.vector.tensor_mul(ta[:, :], D2[:, 0:wid], Bc2[:, 0:wid])
                    nc.vector.tensor_mul(tb[:, :], D2[:, 20 * (p + 1):20 * (p + 1) + wid],
                                         Bc2[:, 20:20 + wid])
                    if p < 3:
                        Bc2 = kpool.tile([128, wid], f32, tag=f"B{p}_2", name="Bc2")
                        nc.vector.tensor_sub(Bc2[:, :], ta[:, :], tb[:, :])
                    else:
                        Bc2 = kpool.tile([128, 161], f32, tag="B3_2", name="Bc2")
                        nc.vector.tensor_sub(Bc2[:, 0:160], ta[:, :], tb[:, :])
                        nc.vector.memset(Bc2[:, 160:161], 1.0)
                stk2a = kpool.tile([108, 128], f32, tag="s2a", name="stk2a")
                stk2b = kpool.tile([53, 128], f32, tag="s2b", name="stk2b")
                ps_b2 = ppk.tile([128, 128], f32, tag="b2", name="ps_b2")
                T.transpose(ps_b2[0:108, 0:128], Bc2[:, 0:108], tcons["iden"][:, :])
                nc.vector.tensor_copy(stk2a[:, :], ps_b2[0:108, 0:128])
                ps_b3 = ppk.tile([128, 128], f32, tag="b3", name="ps_b3")
                T.transpose(ps_b3[0:53, 0:128], Bc2[:, 108:161], tcons["iden"][:, :])
                nc.vector.tensor_copy(stk2b[:, :], ps_b3[0:53, 0:128])
                ps_lg = ppk.tile([128, 128], f32, tag="lg", name="ps_lg")
                T.matmul(ps_lg[0:128, 0:NCLS], stk2a[:, :], tcons["c2b1"][:, :],
                         start=True, stop=False)
                T.matmul(ps_lg[0:128, 0:NCLS], stk2s[:, :], tcons["c2s"][:, :],
                         start=False, stop=False)
                T.matmul(ps_lg[0:128, 0:NCLS], stk2b[:, :], tcons["c2b2"][:, :],
                         start=False, stop=True)
                # ---- log_softmax (on an SBUF copy; ps_lg has 1 PSUM reader) ----
                lg_s = kpool.tile([128, NCLS], f32, tag="lg_s", name="lg_s")
                nc.vector.tensor_copy(lg_s[:, :], ps_lg[0:128, 0:NCLS])
                negm = kpool.tile([128, 1], f32, tag="negm", name="negm")
                nc.vector.reduce_max(negm[:, :], lg_s[:, :], axis=AX.X,
                                     negate=True)
                ex = kpool.tile([128, NCLS], f32, tag="ex", name="ex")
                nc.scalar.activation(ex[:, :], lg_s[:, :], AF.Exp,
                                     bias=negm[:, 0:1])
                ssum = kpool.tile([128, 1], f32, tag="ssum", name="ssum")
                nc.vector.reduce_sum(ssum[:, :], ex[:, :], axis=AX.X)
                lsum = kpool.tile([128, 1], f32, tag="lsum", name="lsum")
                nc.scalar.activation(lsum[:, :], ssum[:, :], AF.Ln,
                                     bias=tcons["misc"][0:128, 2:3])
                res = kpool.tile([128, NCLS], f32, tag="res", name="res")
                nc.vector.tensor_scalar(res[:, :], lg_s[:, :],
                                        negm[:, 0:1], lsum[:, 0:1],
                                        op0=OP.add, op1=OP.subtract)
                nc.sync.dma_start(d_out[sl, :], res[:, :])

    nc.compile()  # bacc lowering: wait splitting via event semaphores, etc.
    _BUILT = (nc, ["xa", "xb"] + list(cons_shapes.keys()))
    return _BUILT


# ----------------------------------------------------------------------------
# entry point
# ----------------------------------------------------------------------------

def kernel(**inputs):
    from concourse import bass_utils

    x = np.asarray(inputs["x"], np.float32)
    cons = _host_consts({k: np.asarray(v, np.float32)
                         for k, v in inputs.items() if k != "x"})
    nc, _names = _build()

    in_maps = []
    for core in range(NCORE):
        xa, xb = _shard_x(x[core * B:(core + 1) * B])
        in_maps.append({"xa": xa, "xb": xb, **cons})
    res = bass_utils.run_bass_kernel_spmd(nc, in_maps, core_ids=list(range(NCORE)))
    return np.concatenate([r["out"] for r in res.results], axis=0)



# revision 9
# speedup vs baseline: 2.5932x; 1.2830x over previous
"""Trainium2 Bass kernel for nn_ChannelWiseConv (depthwise conv stack + KAN head).

Strategy (per core, pure data parallelism over batch):
  - 256 images/core. Each stride-2 depthwise 3x3 conv is computed as 3 PE
    matmuls accumulating in PSUM: contraction over input rows with per-channel
    banded weight matrices (one per kernel-column tap); the column subsampling
    is expressed in the moving-operand access pattern. Channels are blocked
    (2/4/8/13 per matmul as planes shrink) to keep K near 128.
  - bias+relu fused into one ACT/DVE op (PSUM -> SBUF) that also scatters into
    the next layer's channel-blocked input layout.
  - KAN layers: B-spline basis via unnormalized Cox-de-Boor on a uniform grid
    (all denominators folded into host-precomputed coefficient matrices), then
    one PE matmul per KAN layer over [silu(x); basis; 1] stacks.
  - log_softmax on-chip; output [256, 10] per core, concatenated on host.
"""

import ml_dtypes
import numpy as np

BF16 = ml_dtypes.bfloat16
IN_CH, HIDDEN, NCLS = 13, 20, 10
B_FULL, NCORE = 2048, 8
B = B_FULL // NCORE          # images per core
NG = 8                       # image groups per core (DMA granularity)
GI = B // NG                 # images per group (32); PSUM chunks of 16
H_GRID = 0.4                 # KAN knot spacing; u = (x + 2.2) / 0.4

CG1 = [(0, 2), (2, 2), (4, 2), (6, 2), (8, 2), (10, 2), (12, 1)]
CG2 = [(0, 4), (4, 4), (8, 4), (12, 1)]
CG3 = [(0, 8), (8, 5)]

_BUILT = None  # cached (nc, input_names)


# ----------------------------------------------------------------------------
# host-side constant packing
# ----------------------------------------------------------------------------

def _pack_bands(w, S, cgs, slotM, rows):
    So = S // 2
    out = np.zeros((rows, len(cgs) * 3 * slotM), np.float32)
    for gi, (c0, nch) in enumerate(cgs):
        for b in range(3):
            col0 = (gi * 3 + b) * slotM
            for cl in range(nch):
                for i in range(So):
                    for a in range(3):
                        r = 2 * i + a - 1
                        if 0 <= r < S:
                            out[cl * S + r, col0 + cl * So + i] = w[c0 + cl, 0, a, b]
    return out


def _pack_bias(bias, cgs, So, rows):
    out = np.zeros((rows, len(cgs)), np.float32)
    for gi, (c0, nch) in enumerate(cgs):
        for cl in range(nch):
            out[cl * So:(cl + 1) * So, gi] = bias[c0 + cl]
    return out


def _host_consts(inp):
    c = {}
    c["bands1"] = _pack_bands(inp["w1"], 64, CG1, 64, 128).astype(BF16)
    c["bands2"] = _pack_bands(inp["w2"], 32, CG2, 64, 128).astype(BF16)
    c["bands3"] = _pack_bands(inp["w3"], 16, CG3, 64, 128).astype(BF16)
    c["bands4"] = _pack_bands(inp["w4"], 8, [(0, 13)], 52, 104).astype(BF16)
    c["bands5"] = _pack_bands(inp["w5"], 4, [(0, 13)], 26, 52).astype(BF16)
    b6m = np.zeros((26, 26), np.float32)
    for bb in range(2):
        for ch in range(13):
            for r in range(2):
                b6m[ch * 2 + r, bb * 13 + ch] = inp["w6"][ch, 0, r, bb]
    c["bands6"] = b6m.astype(BF16)
    c["bv1"] = _pack_bias(inp["b1"], CG1, 32, 64)
    c["bv2"] = _pack_bias(inp["b2"], CG2, 16, 64)
    c["bv3"] = _pack_bias(inp["b3"], CG3, 8, 64)
    c["bv4"] = _pack_bias(inp["b4"], [(0, 13)], 4, 52)
    c["bv5"] = _pack_bias(inp["b5"], [(0, 13)], 2, 26)
    c["ubias"] = (2.5 * (inp["b6"] + 2.2)).astype(np.float32).reshape(13, 1)

    # KAN layer 1 rhs pieces: silu part [13,20]; basis+bias part [105,20]
    c["c1a"] = np.ascontiguousarray(inp["sb1"].astype(np.float32))
    c1b = np.zeros((105, HIDDEN), np.float32)
    for n in range(8):
        for i in range(13):
            c1b[n * 13 + i] = inp["coef1"][i, :, n] * inp["ss1"][i] / 6.0
    c1b[104] = inp["bias1"]
    c["c1b"] = c1b
    # KAN layer 2 rhs pieces: silu [20,10]; basis rows 0..107 [108,10]; rows 108..159 + bias [53,10]
    c["c2s"] = np.ascontiguousarray(inp["sb2"].astype(np.float32))
    c2b = np.zeros((161, NCLS), np.float32)
    for n in range(8):
        for i in range(20):
            c2b[n * 20 + i] = inp["coef2"][i, :, n] * inp["ss2"][i] / 6.0
    c2b[160] = inp["bias2"]
    c["c2b1"] = np.ascontiguousarray(c2b[0:108])
    c["c2b2"] = np.ascontiguousarray(c2b[108:161])
    misc = np.zeros((128, 3), np.float32)
    misc[:, 0] = -2.2
    misc[:, 1] = 5.5
    c["misc"] = misc
    c["iden"] = np.eye(128, dtype=np.float32)
    return c


def _shard_x(x_shard):
    # [256,13,64,64] -> xa [16,6,128,16,64] (channel pairs), xb [16,64,16,64] (ch 12)
    xs = x_shard.reshape(NG, GI, 13, 64, 64)
    xa = xs[:, :, 0:12].transpose(0, 2, 3, 1, 4).reshape(NG, 6, 128, GI, 64)
    xb = xs[:, :, 12].transpose(0, 2, 1, 3)
    return xa.astype(BF16), xb.astype(BF16)


# ----------------------------------------------------------------------------
# bass program
# ----------------------------------------------------------------------------

def _build():
    global _BUILT
    if _BUILT is not None:
        return _BUILT
    from contextlib import ExitStack
    import concourse.bass as bass  # noqa: F401
    import concourse.bacc as bacc
    import concourse.tile as tile
    import concourse.mybir as mybir

    f32 = mybir.dt.float32
    bf16 = mybir.dt.bfloat16
    AF = mybir.ActivationFunctionType
    OP = mybir.AluOpType
    AX = mybir.AxisListType

    nc = bacc.Bacc("TRN2")
    T = nc.tensor

    d_xa = nc.dram_tensor("xa", [NG, 6, 128, GI, 64], bf16, kind="ExternalInput")
    d_xb = nc.dram_tensor("xb", [NG, 64, GI, 64], bf16, kind="ExternalInput")
    cons_shapes = {
        "bands1": [128, 21 * 64], "bands2": [128, 12 * 64], "bands3": [128, 6 * 64],
        "bands4": [104, 3 * 52], "bands5": [52, 3 * 26], "bands6": [26, 26],
        "bv1": [64, 7], "bv2": [64, 4], "bv3": [64, 2], "bv4": [52, 1],
        "bv5": [26, 1], "ubias": [13, 1], "misc": [128, 3], "c1a": [13, 20],
        "c1b": [105, 20], "c2s": [20, 10], "c2b1": [108, 10], "c2b2": [53, 10],
        "iden": [128, 128],
    }
    BANDS = {"bands1", "bands2", "bands3", "bands4", "bands5", "bands6"}
    d_cons = {k: nc.dram_tensor(k, v, bf16 if k in BANDS else f32,
                                kind="ExternalInput")
              for k, v in cons_shapes.items()}
    d_out = nc.dram_tensor("out", [B, NCLS], f32, kind="ExternalOutput")

    with tile.TileContext(nc) as tc, ExitStack() as ctx:
        cpool = ctx.enter_context(tc.tile_pool(name="consts", bufs=1))
        tcons = {}
        for k, shp in cons_shapes.items():
            t = cpool.tile(shp, bf16 if k in BANDS else f32, name=f"t_{k}")
            nc.sync.dma_start(t[:, :], d_cons[k][:, :])
            tcons[k] = t
        t_u = cpool.tile([13, B], f32, name="t_u")

        p_x1 = ctx.enter_context(tc.tile_pool(name="x1", bufs=3))
        p_x2 = ctx.enter_context(tc.tile_pool(name="x2", bufs=2))
        p_x3 = ctx.enter_context(tc.tile_pool(name="x3", bufs=2))
        p_sm = ctx.enter_context(tc.tile_pool(name="xsm", bufs=2))

        with tc.tile_pool(name="ps1", bufs=2, space="PSUM") as pp1, \
             tc.tile_pool(name="ps2", bufs=2, space="PSUM") as pp2, \
             tc.tile_pool(name="ps3", bufs=1, space="PSUM") as pp3, \
             tc.tile_pool(name="ps456", bufs=2, space="PSUM") as pp456:
            # PE observes each const DMA once, so real matmuls (which allow
            # only a single attached sync-wait on the LDWEIGHTS) never need
            # a const-DMA wait on top of their data wait.
            scrap = pp1.tile([1, 16], f32, tag="scrap", bufs=1, name="scrap")
            for k in ("bands1", "bands2", "bands3", "bands4", "bands5",
                      "bands6", "c1a", "c1b", "c2s", "c2b1", "c2b2", "iden"):
                T.matmul(scrap[0:1, 0:1], tcons[k][0:1, 0:1],
                         tcons[k][0:1, 0:1], start=True, stop=True)
            # bias+relu (PSUM->SBUF drain) split across scalar+vector engines
            # (GPSIMD cannot read PSUM)
            def relu_bias(eng, dest, src, bap):
                if eng == 0:
                    nc.scalar.activation(dest, src, AF.Relu, bias=bap)
                else:
                    nc.vector.tensor_scalar(dest, src, bap, 0.0,
                                            op0=OP.add, op1=OP.max)

            E1 = [0, 1, 0, 1, 0, 1, 0]
            E2 = [0, 1, 0, 1]
            E3 = [0, 1]

            for g in range(NG):
                # ---- conv1 + scatter into 4ch-blocked x2 tiles ----
                x2t = [p_x2.tile([128, GI * 32], bf16, tag=f"x2_{k}", name=f"x2_{k}")
                       for k in range(4)]
                for cg, (c0, nch) in enumerate(CG1):
                    K, M = nch * 64, nch * 32
                    xt = p_x1.tile([128, GI * 64], bf16, tag="x1", name="xt")
                    if nch == 2:
                        nc.sync.dma_start(
                            xt[:, :].rearrange("p (i w) -> p i w", w=64),
                            d_xa[g, cg, :, :, :])
                    else:
                        nc.sync.dma_start(
                            xt[0:64, :].rearrange("p (i w) -> p i w", w=64),
                            d_xb[g, :, :, :])
                    # absorb the input-DMA wait into a scrap matmul so the
                    # real matmuls carry only the PSUM-slot (ACT) wait
                    T.matmul(scrap[0:1, 0:1], xt[0:1, 0:1], xt[0:1, 0:1],
                             start=True, stop=True)
                    xv = xt[0:K, :].rearrange("p (i w) -> p i w", w=64)
                    lo = lambda b: (cg * 3 + b) * 64
                    for h in range(2):
                        hs = slice(16 * h, 16 * h + 16)
                        ps = pp1.tile([64, 512], f32, tag="ps1", name="ps1t")
                        pv = ps[0:M, :].rearrange("p (i w) -> p i w", w=32)
                        T.matmul(pv, tcons["bands1"][0:K, lo(1):lo(1) + M],
                                 xv[:, hs, 0:64:2], start=True, stop=False)
                        T.matmul(pv, tcons["bands1"][0:K, lo(2):lo(2) + M],
                                 xv[:, hs, 1:64:2], start=False, stop=False)
                        T.matmul(pv[:, :, 1:32], tcons["bands1"][0:K, lo(0):lo(0) + M],
                                 xv[:, hs, 1:62:2], start=False, stop=True,
                                 skip_group_check=True)
                        dest = x2t[cg // 2][64 * (cg % 2):64 * (cg % 2) + M,
                                            h * 512:h * 512 + 512]
                        bap = tcons["bv1"][0:M, cg:cg + 1]
                        relu_bias(E1[cg], dest, ps[0:M, :], bap)

                # ---- conv2 -> x3 tiles (8ch-blocked) ----
                x3t = [p_x3.tile([128, GI * 16], bf16, tag=f"x3_{k}", name=f"x3_{k}")
                       for k in range(2)]
                for k4, (c0, nch) in enumerate(CG2):
                    K, M = nch * 32, nch * 16
                    xv = x2t[k4][0:K, :].rearrange("p (i w) -> p i w", w=32)
                    lo = lambda b: (k4 * 3 + b) * 64
                    for h in range(2):
                        hs = slice(16 * h, 16 * h + 16)
                        ps = pp2.tile([64, 256], f32, tag="ps2", name="ps2t")
                        pv = ps[0:M, :].rearrange("p (i w) -> p i w", w=16)
                        T.matmul(pv, tcons["bands2"][0:K, lo(1):lo(1) + M],
                                 xv[:, hs, 0:32:2], start=True, stop=False)
                        T.matmul(pv, tcons["bands2"][0:K, lo(2):lo(2) + M],
                                 xv[:, hs, 1:32:2], start=False, stop=False)
                        T.matmul(pv[:, :, 1:16], tcons["bands2"][0:K, lo(0):lo(0) + M],
                                 xv[:, hs, 1:30:2], start=False, stop=True,
                                 skip_group_check=True)
                        dest = x3t[k4 // 2][64 * (k4 % 2):64 * (k4 % 2) + M,
                                            h * 256:h * 256 + 256]
                        bap = tcons["bv2"][0:M, k4:k4 + 1]
                        relu_bias(E2[k4], dest, ps[0:M, :], bap)

                # ---- conv3 -> x4 [104, GI*8] ----
                x4 = p_sm.tile([104, GI * 8], bf16, tag="x4", name="x4")
                for k8, (c0, nch) in enumerate(CG3):
                    K, M = nch * 16, nch * 8
                    xv = x3t[k8][0:K, :].rearrange("p (i w) -> p i w", w=16)
                    lo = lambda b: (k8 * 3 + b) * 64
                    for h in range(2):
                        hs = slice(16 * h, 16 * h + 16)
                        ps = pp3.tile([64, 128], f32, tag="ps3", name="ps3t")
                        pv = ps[0:M, :].rearrange("p (i w) -> p i w", w=8)
                        T.matmul(pv, tcons["bands3"][0:K, lo(1):lo(1) + M],
                                 xv[:, hs, 0:16:2], start=True, stop=False)
                        T.matmul(pv, tcons["bands3"][0:K, lo(2):lo(2) + M],
                                 xv[:, hs, 1:16:2], start=False, stop=False)
                        T.matmul(pv[:, :, 1:8], tcons["bands3"][0:K, lo(0):lo(0) + M],
                                 xv[:, hs, 1:14:2], start=False, stop=True,
                                 skip_group_check=True)
                        dest = x4[64 * k8:64 * k8 + M, h * 128:h * 128 + 128]
                        bap = tcons["bv3"][0:M, k8:k8 + 1]
                        relu_bias(E3[k8], dest, ps[0:M, :], bap)

                # ---- conv4 [104 -> 52] ----
                x5 = p_sm.tile([52, GI * 4], bf16, tag="x5", name="x5")
                xv = x4[0:104, :].rearrange("p (i w) -> p i w", w=8)
                for h in range(2):
                    hs = slice(16 * h, 16 * h + 16)
                    ps4 = pp456.tile([64, 64], f32, tag="ps456", name="ps4t")
                    pv = ps4[0:52, :].rearrange("p (i w) -> p i w", w=4)
                    T.matmul(pv, tcons["bands4"][0:104, 52:104], xv[:, hs, 0:8:2],
                             start=True, stop=False)
                    T.matmul(pv, tcons["bands4"][0:104, 104:156], xv[:, hs, 1:8:2],
                             start=False, stop=False)
                    T.matmul(pv[:, :, 1:4], tcons["bands4"][0:104, 0:52],
                             xv[:, hs, 1:6:2], start=False, stop=True,
                             skip_group_check=True)
                    nc.scalar.activation(x5[:, h * 64:h * 64 + 64], ps4[0:52, :],
                                         AF.Relu, bias=tcons["bv4"][0:52, 0:1])

                # ---- conv5 [52 -> 26] ----
                x6 = p_sm.tile([26, GI * 2], bf16, tag="x6", name="x6")
                xv = x5[0:52, :].rearrange("p (i w) -> p i w", w=4)
                for h in range(2):
                    hs = slice(16 * h, 16 * h + 16)
                    ps5 = pp456.tile([64, 32], f32, tag="ps456", name="ps5t")
                    pv = ps5[0:26, :].rearrange("p (i w) -> p i w", w=2)
                    T.matmul(pv, tcons["bands5"][0:52, 26:52], xv[:, hs, 0:4:2],
                             start=True, stop=False)
                    T.matmul(pv, tcons["bands5"][0:52, 52:78], xv[:, hs, 1:4:2],
                             start=False, stop=False)
                    T.matmul(pv[:, :, 1:2], tcons["bands5"][0:52, 0:26],
                             xv[:, hs, 1:2:2], start=False, stop=True,
                             skip_group_check=True)
                    nc.scalar.activation(x6[:, h * 32:h * 32 + 32], ps5[0:26, :],
                                         AF.Relu, bias=tcons["bv5"][0:26, 0:1])

                # ---- conv6 (2x2 valid) -> u[:, g*GI : g*GI+GI] ----
                xv = x6[0:26, :].rearrange("p (i w) -> p i w", w=2)
                for h in range(2):
                    hs = slice(16 * h, 16 * h + 16)
                    ps6 = pp456.tile([64, 16], f32, tag="ps456", name="ps6t")
                    T.matmul(ps6[0:13, :], tcons["bands6"][0:26, 0:13],
                             xv[:, hs, 0:1], start=True, stop=False)
                    T.matmul(ps6[0:13, :], tcons["bands6"][0:26, 13:26],
                             xv[:, hs, 1:2], start=False, stop=True)
                    nc.scalar.activation(t_u[:, g * GI + 16 * h:g * GI + 16 * h + 16],
                                         ps6[0:13, :], AF.Identity,
                                         bias=tcons["ubias"][0:13, 0:1], scale=2.5)

        # ------------------------------ KAN head ------------------------------
        kpool = ctx.enter_context(tc.tile_pool(name="kan", bufs=2))
        with tc.tile_pool(name="psk", bufs=1, space="PSUM") as ppk:
            for t in range(B // 128):
                sl = slice(t * 128, (t + 1) * 128)
                # uT [128, 13] via PE transpose
                ps_uT = ppk.tile([128, 128], f32, tag="uT", name="ps_uT")
                T.transpose(ps_uT[0:128, 0:13], t_u[0:13, sl], tcons["iden"][0:13, 0:13])
                # D[:, k*13 : k*13+13] = u - k
                D = kpool.tile([128, 156], f32, tag="D", name="Dt")
                for k in range(12):
                    nc.vector.tensor_scalar(D[:, k * 13:(k + 1) * 13],
                                            ps_uT[0:128, 0:13], float(-k), None,
                                            op0=OP.add)
                # degree-0 basis
                ge = kpool.tile([128, 143], f32, tag="ge", name="ge")
                lt = kpool.tile([128, 143], f32, tag="lt", name="lt")
                nc.vector.tensor_scalar(ge[:, :], D[:, 0:143], 0.0, None, op0=OP.is_ge)
                nc.vector.tensor_scalar(lt[:, :], D[:, 13:156], 0.0, None, op0=OP.is_lt)
                Bc = kpool.tile([128, 143], f32, tag="B0", name="Bc")
                nc.vector.tensor_mul(Bc[:, :], ge[:, :], lt[:, :])
                # Cox-de-Boor levels (unnormalized; /6 folded into c1b)
                wid = 143
                for p in range(1, 4):
                    wid -= 13
                    ta = kpool.tile([128, wid], f32, tag=f"ta{p}", name="ta")
                    tb = kpool.tile([128, wid], f32, tag=f"tb{p}", name="tb")
                    nc.vector.tensor_mul(ta[:, :], D[:, 0:wid], Bc[:, 0:wid])
                    nc.vector.tensor_mul(tb[:, :], D[:, 13 * (p + 1):13 * (p + 1) + wid],
                                         Bc[:, 13:13 + wid])
                    if p < 3:
                        Bc = kpool.tile([128, wid], f32, tag=f"B{p}", name="Bc")
                        nc.vector.tensor_sub(Bc[:, :], ta[:, :], tb[:, :])
                    else:
                        Bc = kpool.tile([128, 105], f32, tag="B3", name="Bc")
                        nc.vector.tensor_sub(Bc[:, 0:104], ta[:, :], tb[:, :])
                        nc.vector.memset(Bc[:, 104:105], 1.0)
                # stacks: silu part [13,128]; (basis;1)^T part [105,128]
                stkA = kpool.tile([13, 128], f32, tag="stkA", name="stkA")
                stkB = kpool.tile([105, 128], f32, tag="stkB", name="stkB")
                ps_b1 = ppk.tile([128, 128], f32, tag="b1", name="ps_b1")
                T.transpose(ps_b1[0:105, 0:128], Bc[:, 0:105], tcons["iden"][:, :])
                nc.vector.tensor_copy(stkB[:, :], ps_b1[0:105, 0:128])
                nc.scalar.activation(stkA[:, :], t_u[0:13, sl], AF.Silu,
                                     bias=tcons["misc"][0:13, 0:1], scale=H_GRID)
                ps_h1 = ppk.tile([128, 128], f32, tag="h1", name="ps_h1")
                T.matmul(ps_h1[0:128, 0:20], stkA[:, :], tcons["c1a"][:, :],
                         start=True, stop=False)
                T.matmul(ps_h1[0:128, 0:20], stkB[:, :], tcons["c1b"][:, :],
                         start=False, stop=True)
                # ---- KAN layer 2 ----
                u2 = kpool.tile([128, 20], f32, tag="u2", name="u2")
                nc.scalar.activation(u2[:, :], ps_h1[0:128, 0:20], AF.Identity,
                                     bias=tcons["misc"][0:128, 1:2], scale=2.5)
                stk2s = kpool.tile([20, 128], f32, tag="s2s", name="stk2s")
                ps_t2 = ppk.tile([128, 128], f32, tag="t2", name="ps_t2")
                T.transpose(ps_t2[0:20, 0:128], u2[:, :], tcons["iden"][:, :])
                nc.scalar.activation(stk2s[:, :], ps_t2[0:20, 0:128], AF.Silu,
                                     bias=tcons["misc"][0:20, 0:1], scale=H_GRID)
                D2 = kpool.tile([128, 240], f32, tag="D2", name="D2t")
                for k in range(12):
                    nc.vector.tensor_scalar(D2[:, k * 20:(k + 1) * 20], u2[:, :],
                                            float(-k), None, op0=OP.add)
                ge2 = kpool.tile([128, 220], f32, tag="ge2", name="ge2")
                lt2 = kpool.tile([128, 220], f32, tag="lt2", name="lt2")
                nc.vector.tensor_scalar(ge2[:, :], D2[:, 0:220], 0.0, None, op0=OP.is_ge)
                nc.vector.tensor_scalar(lt2[:, :], D2[:, 20:240], 0.0, None, op0=OP.is_lt)
                Bc2 = kpool.tile([128, 220], f32, tag="B0_2", name="Bc2")
                nc.vector.tensor_mul(Bc2[:, :], ge2[:, :], lt2[:, :])
                wid = 220
                for p in range(1, 4):
                    wid -= 20
                    ta = kpool.tile([128, wid], f32, tag=f"t2a{p}", name="ta2")
                    tb = kpool.tile([128, wid], f32, tag=f"t2b{p}", name="tb2")
                    nc.vector.tensor_mul(ta[:, :], D2[:, 0:wid], Bc2[:, 0:wid])
                    nc.vector.tensor_mul(tb[:, :], D2[:, 20 * (p + 1):20 * (p + 1) + wid],
                                         Bc2[:, 20:20 + wid])
                    if p < 3:
                        Bc2 = kpool.tile([128, wid], f32, tag=f"B{p}_2", name="Bc2")
                        nc.vector.tensor_sub(Bc2[:, :], ta[:, :], tb[:, :])
                    else:
                        Bc2 = kpool.tile([128, 161], f32, tag="B3_2", name="Bc2")
                        nc.vector.tensor_sub(Bc2[:, 0:160], ta[:, :], tb[:, :])
                        nc.vector.memset(Bc2[:, 160:161], 1.0)
                stk2a = kpool.tile([108, 128], f32, tag="s2a", name="stk2a")
                stk2b = kpool.tile([53, 128], f32, tag="s2b", name="stk2b")
                ps_b2 = ppk.tile([128, 128], f32, tag="b2", name="ps_b2")
                T.transpose(ps_b2[0:108, 0:128], Bc2[:, 0:108], tcons["iden"][:, :])
                nc.vector.tensor_copy(stk2a[:, :], ps_b2[0:108, 0:128])
                ps_b3 = ppk.tile([128, 128], f32, tag="b3", name="ps_b3")
                T.transpose(ps_b3[0:53, 0:128], Bc2[:, 108:161], tcons["iden"][:, :])
                nc.vector.tensor_copy(stk2b[:, :], ps_b3[0:53, 0:128])
                ps_lg = ppk.tile([128, 128], f32, tag="lg", name="ps_lg")
                T.matmul(ps_lg[0:128, 0:NCLS], stk2a[:, :], tcons["c2b1"][:, :],
                         start=True, stop=False)
                T.matmul(ps_lg[0:128, 0:NCLS], stk2s[:, :], tcons["c2s"][:, :],
                         start=False, stop=False)
                T.matmul(ps_lg[0:128, 0:NCLS], stk2b[:, :], tcons["c2b2"][:, :],
                         start=False, stop=True)
                # ---- log_softmax (on an SBUF copy; ps_lg has 1 PSUM reader) ----
                lg_s = kpool.tile([128, NCLS], f32, tag="lg_s", name="lg_s")
                nc.vector.tensor_copy(lg_s[:, :], ps_lg[0:128, 0:NCLS])
                negm = kpool.tile([128, 1], f32, tag="negm", name="negm")
                nc.vector.reduce_max(negm[:, :], lg_s[:, :], axis=AX.X,
                                     negate=True)
                ex = kpool.tile([128, NCLS], f32, tag="ex", name="ex")
                nc.scalar.activation(ex[:, :], lg_s[:, :], AF.Exp,
                                     bias=negm[:, 0:1])
                ssum = kpool.tile([128, 1], f32, tag="ssum", name="ssum")
                nc.vector.reduce_sum(ssum[:, :], ex[:, :], axis=AX.X)
                lsum = kpool.tile([128, 1], f32, tag="lsum", name="lsum")
                nc.scalar.activation(lsum[:, :], ssum[:, :], AF.Ln,
                                     bias=tcons["misc"][0:128, 2:3])
                res = kpool.tile([128, NCLS], f32, tag="res", name="res")
                nc.vector.tensor_scalar(res[:, :], lg_s[:, :],
                                        negm[:, 0:1], lsum[:, 0:1],
                                        op0=OP.add, op1=OP.subtract)
                nc.sync.dma_start(d_out[sl, :], res[:, :])

    nc.compile()  # bacc lowering: wait splitting via event semaphores, etc.
    _BUILT = (nc, ["xa", "xb"] + list(cons_shapes.keys()))
    return _BUILT


# ----------------------------------------------------------------------------
# entry point
# ----------------------------------------------------------------------------

def kernel(**inputs):
    from concourse import bass_utils

    x = np.asarray(inputs["x"], np.float32)
    cons = _host_consts({k: np.asarray(v, np.float32)
                         for k, v in inputs.items() if k != "x"})
    nc, _names = _build()

    in_maps = []
    for core in range(NCORE):
        xa, xb = _shard_x(x[core * B:(core + 1) * B])
        in_maps.append({"xa": xa, "xb": xb, **cons})
    res = bass_utils.run_bass_kernel_spmd(nc, in_maps, core_ids=list(range(NCORE)))
    return np.concatenate([r["out"] for r in res.results], axis=0)

